# revision 43
# baseline (speedup 1.0000x reference)
"""Multi-head causal attention (B=4, L=2048, D=1024, H=16) on 8 trn2 cores.

Sharding: (batch, head-group) grid — core c handles batch c//2, heads
(c%2)*8..(c%2)*8+8.  Each core projects Q/K/V for its 8 heads, runs causal
attention, and computes a partial output projection; the host sums the two
head-group partials per batch.

Per-core layouts (host prepares transposed inputs so every matmul contracts
over the partition dim):
  xq_t/xk_t/xv_t [D, L]   : x.T            (rhs / lhsT of projections)
  wq_t/wk_t/wv_t [D, 512] : W_slice.T      (wq pre-scaled by 1/sqrt(dh))
  wo_t           [512, D] : Wo_slice.T
  qT/kT pair tiles [128, L]: rows 0-63 head 2p, 64-127 head 2p+1 (dh on P)
  v_aug [128, 8, 65]      : per 128-token chunk; [:, h, 0:64]=V, [:, h, 64]=key mask
  scores ST [k(P), q(F)]  : transposed scores -> softmax sum via matmul's
                            extra mask column (pv row 64), no P-transposes.

Schedule: projections, attention sweeps and output-projection chunks are
woven into one issue stream so the Scalar engine's EXP (the secondary
bottleneck) always overlaps PE matmuls.  Attention runs as per-(qb, pair)
sweeps with lag-1 PV issue; proj/wo units are inserted as PE filler inside
each sweep.  PSUM: 2x score double-buffer (4 banks) + 3 pv accumulators +
1 proj/wo bank = 8.
"""

import math
from contextlib import ExitStack

import numpy as np

import concourse.bass as bass
import concourse.tile as tile
from concourse import bacc, mybir
from concourse import bass_utils

D = 1024  # model dim
HG = 512  # head dims per core (8 heads x 64)
NH = 8    # heads per core
DH = 64
NPAIR = 4  # head pairs per core
NEG = -1.0e30

F32 = mybir.dt.float32
BF16 = mybir.dt.bfloat16
EXP = mybir.ActivationFunctionType.Exp
COPY = mybir.ActivationFunctionType.Copy


def build(L=2048):
    TQ = L // 512    # 512-token q-blocks
    T16 = L // 128   # 128-token chunks
    DCH = D // 128   # contraction chunks for projections

    nc = bacc.Bacc("TRN2", target_bir_lowering=False, debug=False, num_devices=8)

    xq = nc.dram_tensor("xq_t", [D, L], BF16, kind="ExternalInput").ap()
    xk = nc.dram_tensor("xk_t", [D, L], BF16, kind="ExternalInput").ap()
    xv = nc.dram_tensor("xv_t", [D, L], BF16, kind="ExternalInput").ap()
    wq = nc.dram_tensor("wq_t", [D, HG], BF16, kind="ExternalInput").ap()
    wk = nc.dram_tensor("wk_t", [D, HG], BF16, kind="ExternalInput").ap()
    wv = nc.dram_tensor("wv_t", [D, HG], BF16, kind="ExternalInput").ap()
    wo = nc.dram_tensor("wo_t", [HG, D], BF16, kind="ExternalInput").ap()
    mcol = nc.dram_tensor("maskcol", [128, (L // 128) * NH], F32, kind="ExternalInput").ap()
    trim = nc.dram_tensor("trimask", [128, 128], BF16, kind="ExternalInput").ap()
    iden = nc.dram_tensor("ident", [128, 128], BF16, kind="ExternalInput").ap()
    outp = nc.dram_tensor("outp", [L, D], F32, kind="ExternalOutput").ap()

    with ExitStack() as ctx:
        tc = ctx.enter_context(tile.TileContext(nc))

        # ---- persistent tiles ----
        singles = ctx.enter_context(tc.tile_pool(name="singles", bufs=1))
        qT = [singles.tile([128, L], BF16, tag=f"qT{p}", name=f"qT{p}") for p in range(NPAIR)]
        kT = [singles.tile([128, L], BF16, tag=f"kT{p}", name=f"kT{p}") for p in range(NPAIR)]
        vaug = [singles.tile([128, NH, DH + 1], BF16, tag=f"vaug{t}", name=f"vaug{t}") for t in range(T16)]
        ctxT = [singles.tile([128, L], BF16, tag=f"ctxT{p}", name=f"ctxT{p}") for p in range(NPAIR)]
        mc_sb = singles.tile([128, T16, NH], F32, tag="mc")
        tri_sb = singles.tile([128, 128], BF16, tag="tri")
        id_sb = singles.tile([128, 128], BF16, tag="ident")

        nc.sync.dma_start(out=mc_sb, in_=mcol.rearrange("p (t h) -> p t h", h=NH))
        nc.sync.dma_start(out=tri_sb, in_=trim)
        nc.sync.dma_start(out=id_sb, in_=iden)

        with (
            tc.tile_pool(name="xt", bufs=26) as xtp,
            tc.tile_pool(name="w", bufs=3 * DCH) as wp,
            tc.tile_pool(name="stp", bufs=2, space="PSUM") as stp,
            tc.tile_pool(name="pvp", bufs=3, space="PSUM") as pvp,
            tc.tile_pool(name="projp", bufs=1, space="PSUM") as projp,
            tc.tile_pool(name="expp", bufs=5) as expp,
            tc.tile_pool(name="recp", bufs=2) as recp,
            tc.tile_pool(name="bcsb", bufs=2) as bcsbp,
            tc.tile_pool(name="tmpb", bufs=4) as tmpbp,
            tc.tile_pool(name="wop", bufs=NPAIR) as wop,
            tc.tile_pool(name="outp_sb", bufs=3) as outsb,
        ):
            # PE warm-up (HAM) while the first DMAs land: a few dummy
            # matmuls with no data dependencies.
            wu = singles.tile([128, 256], BF16, tag="warm")
            nc.vector.memset(wu, 0.0)
            ones = singles.tile([128, DH], F32, tag="ones")
            nc.vector.memset(ones, 1.0)
            wups = stp.tile([128, 2, 512], F32, tag="st", name="wupstile")
            for _ in range(38):
                nc.tensor.matmul(
                    wups[:, 0, 0:256], lhsT=wu[:, 0:128], rhs=wu, start=True, stop=True
                )

            def load_w(wdram, eng=None):
                eng = eng or nc.sync
                tiles = [wp.tile([128, HG], BF16, tag="w", name="wtile") for _ in range(DCH)]
                for d in range(DCH):
                    eng.dma_start(out=tiles[d], in_=wdram[d * 128:(d + 1) * 128, :])
                return tiles

            xts_cache = {}

            def _xts_entry(which, t):
                key = (which, t)
                if key not in xts_cache:
                    xts_cache[key] = {
                        "tiles": [xtp.tile([128, 512], BF16, tag="xt", name="xtile")
                                  for _ in range(DCH)],
                        "issued": [False] * DCH,
                    }
                return xts_cache[key]

            def _issue_xd(which, t, d, eng):
                ent = _xts_entry(which, t)
                if not ent["issued"][d]:
                    xd = {"q": xq, "k": xk, "v": xv}[which]
                    eng.dma_start(
                        out=ent["tiles"][d],
                        in_=xd[d * 128:(d + 1) * 128, t * 512:(t + 1) * 512],
                    )
                    ent["issued"][d] = True

            def get_xts(which, t, eng=None):
                e = eng or nc.sync
                for d in range(DCH):
                    _issue_xd(which, t, d, e)
                return _xts_entry(which, t)["tiles"]

            def prefetch_unit(which, t, d):
                # one tile's input DMA, woven into the filler stream so the
                # descriptor burst never floods all queues at once
                def run():
                    _issue_xd(which, t, d, nc.sync)
                return run

            wtiles = {}

            # ---- unit builders (each returns a closure doing ~1-2us of PE) ----
            def kq_unit(which, dst, t, m, evac, pool=None):
                # dst[m][:, t*512:+512] = (W.T chunk m).T @ xT
                def run():
                    xts = get_xts(which, t)
                    wt = wtiles[which]
                    pl = pool or projp
                    ps = pl.tile([128, 512], F32, tag="pp" if pl is projp else "pv",
                                 name="psproj")
                    for d in range(DCH):
                        nc.tensor.matmul(
                            ps,
                            lhsT=wt[d][:, m * 128:(m + 1) * 128],
                            rhs=xts[d],
                            start=(d == 0),
                            stop=(d == DCH - 1),
                        )
                    dstap = dst[m][:, t * 512:(t + 1) * 512]
                    if evac == "act":
                        nc.scalar.copy(dstap, ps)
                    else:
                        nc.vector.tensor_copy(dstap, ps)
                return run

            def v_unit(t, s, evac, pool=None):
                # v_aug[t16][:, h, 0:64] = (x @ Wv.T)[tok chunk, head h] * mask
                def run():
                    xts = get_xts("v", t)
                    wt = wtiles["v"]
                    t16 = t * 4 + s
                    pl = pool or projp
                    ps = pl.tile([128, 512], F32, tag="pp" if pl is projp else "pv",
                                 name="psv")
                    for d in range(DCH):
                        nc.tensor.matmul(
                            ps,
                            lhsT=xts[d][:, s * 128:(s + 1) * 128],
                            rhs=wt[d],
                            start=(d == 0),
                            stop=(d == DCH - 1),
                        )
                    src = ps.rearrange("p (h e) -> p h e", h=NH)
                    dstap = vaug[t16][:, :, 0:DH]
                    if evac == "act":
                        nc.scalar.activation(
                            out=dstap, in_=src, func=COPY,
                            scale=mc_sb[:, t16, 0:1],
                        )
                    else:
                        nc.vector.tensor_scalar_mul(dstap, src, mc_sb[:, t16, 0:1])
                    nc.vector.tensor_copy(
                        vaug[t16][:, :, DH:DH + 1], mc_sb[:, t16:t16 + 1, :],
                    )
                return run

            wo_sb = [wop.tile([128, D], BF16, tag="wo", name="wotile") for _ in range(NPAIR)]

            def load_wo():
                for c in range(NPAIR):
                    nc.sync.dma_start(out=wo_sb[c], in_=wo[c * 128:(c + 1) * 128, :])

            def wo_unit(t16, evac="dve"):
                def run():
                    ot = outsb.tile([128, D], F32, tag="ot", name="ottile")
                    for oh in range(2):
                        ps = projp.tile([128, 512], F32, tag="pp", name="potile")
                        for c in range(NPAIR):
                            nc.tensor.matmul(
                                ps,
                                lhsT=ctxT[c][:, t16 * 128:(t16 + 1) * 128],
                                rhs=wo_sb[c][:, oh * 512:(oh + 1) * 512],
                                start=(c == 0),
                                stop=(c == NPAIR - 1),
                            )
                        oap = ot[:, oh * 512:(oh + 1) * 512]
                        if evac == "act":
                            nc.scalar.copy(oap, ps)
                        else:
                            nc.vector.tensor_copy(oap, ps)
                    nc.sync.dma_start(out=outp[t16 * 128:(t16 + 1) * 128, :], in_=ot)
                return run

            # ---- attention sweep over one (qb, pair): list of units with
            # lag-1 PV issue so the PE never waits at an EXP-gated matmul.
            def sweep_units(qb, p):
                nkc = 4 * (qb + 1)
                pv = [pvp.tile([DH + 1, 512], F32, tag="pv", name="pvtile")
                      for _ in range(2)]
                exs = {}

                def score_unit(kc):
                    def run():
                        j = kc - 4 * qb
                        off = j * 128 if j >= 0 else 0
                        st = stp.tile([128, 2, 512], F32, tag="st", name="sttile")
                        for ph in range(2):
                            nc.tensor.matmul(
                                st[:, ph, off:512],
                                lhsT=kT[p][ph * DH:(ph + 1) * DH,
                                           kc * 128:(kc + 1) * 128],
                                rhs=qT[p][ph * DH:(ph + 1) * DH,
                                          qb * 512 + off:(qb + 1) * 512],
                                start=True,
                                stop=(j < 0),
                            )
                        if j >= 0:
                            # causal mask via PE: st += I.T @ tri
                            for ph in range(2):
                                nc.tensor.matmul(
                                    st[:, ph, off:off + 128],
                                    lhsT=id_sb,
                                    rhs=tri_sb,
                                    start=False,
                                    stop=True,
                                )
                        ex = expp.tile([128, 2, 512], BF16, tag="ex", name="extile")
                        nc.scalar.activation(
                            out=ex[:, :, off:512], in_=st[:, :, off:512], func=EXP
                        )
                        exs[kc] = (ex, off)
                    return run

                def pv_unit(kc):
                    def run():
                        ex, off = exs.pop(kc)
                        for ph in range(2):
                            nc.tensor.matmul(
                                pv[ph][:, off:512],
                                lhsT=vaug[kc][:, 2 * p + ph, :],
                                rhs=ex[:, ph, off:512],
                                start=(kc == 0),
                                stop=(kc == nkc - 1),
                            )
                    return run

                nstate = {}
                last_sweep = (qb == 3 and p == 3)

                def norm_a():
                    def run():
                        bcs = bcsbp.tile([DH, 2, 512], F32, tag="bcs", name="bcstile")
                        if last_sweep:
                            # latency-critical: PE ones-outer-product
                            # broadcast + wide reciprocal
                            s64 = recp.tile([128, 2, 512], F32, tag="rec", name="s64tile")
                            for ph in range(2):
                                nc.vector.tensor_copy(
                                    s64[DH:DH + 1, ph, :], pv[ph][DH:DH + 1, :]
                                )
                            dps = stp.tile([128, 2, 512], F32, tag="st", name="dbcast")
                            for ph in range(2):
                                nc.tensor.matmul(
                                    dps[0:DH, ph, :],
                                    lhsT=ones[DH:DH + 1, :],
                                    rhs=s64[DH:DH + 1, ph, :],
                                    start=True,
                                    stop=True,
                                )
                            nc.vector.reciprocal_approx_fast(
                                out=bcs, in_=dps[0:DH, :, :]
                            )
                        else:
                            # two per-ph chains pipelined across DVE/GPS:
                            # copy -> DMA to partition 0 -> reciprocal ->
                            # GPSIMD broadcast
                            for ph in range(2):
                                s64 = recp.tile([128, 512], F32, tag="rec", name="s64tile")
                                nc.vector.tensor_copy(
                                    s64[DH:DH + 1, :], pv[ph][DH:DH + 1, :]
                                )
                                srow = recp.tile([1, 512], F32, tag="srow", name="srowtile")
                                nc.gpsimd.dma_start(out=srow, in_=s64[DH:DH + 1, :])
                                rcp = recp.tile([1, 512], F32, tag="rcp", name="rcptile")
                                nc.vector.reciprocal_approx_fast(out=rcp, in_=srow)
                                nc.gpsimd.partition_broadcast(
                                    bcs[:, ph, :], rcp, channels=DH
                                )
                        nstate["bcs"] = bcs
                    return run

                def norm_b():
                    def run():
                        # ctxT[p][:, qb block] = pv[0:64] * (1/denom); ph0
                        # first — its slot gates the next sweep's PV via the
                        # psum pool ring
                        bcs = nstate.pop("bcs")
                        nc.vector.tensor_mul(
                            ctxT[p][0:DH, qb * 512:(qb + 1) * 512],
                            pv[0][0:DH, :],
                            bcs[:, 0, :],
                        )
                        tb = tmpbp.tile([DH, 512], BF16, tag="tb", name="tbtile")
                        nc.vector.tensor_mul(tb, pv[1][0:DH, :], bcs[:, 1, :])
                        shift_eng = nc.sync if last_sweep else nc.gpsimd
                        shift_eng.dma_start(
                            out=ctxT[p][DH:128, qb * 512:(qb + 1) * 512],
                            in_=tb,
                        )
                    return run

                def gap_dummy(n):
                    # dummy matmuls in the proj/wo psum slot: execute while
                    # this sweep's first PV waits on the pv-ring WAR, keeping
                    # the HAM activity window busy (qb0 sweeps are shorter
                    # than the norm chain)
                    def run():
                        ps = projp.tile([128, 512], F32, tag="pp", name="gapdummy")
                        for _ in range(n):
                            nc.tensor.matmul(
                                ps, lhsT=wu[:, 0:128], rhs=qT[0][:, 0:512],
                                start=True, stop=True,
                            )
                    return run

                units = [score_unit(0)]
                for kc in range(1, nkc):
                    units.append(score_unit(kc))
                    units.append(pv_unit(kc - 1))
                units.append(pv_unit(nkc - 1))
                units.append(norm_a())
                units.append(norm_b())
                return units

            def weave(attn, filler):
                # filler: list of (unit, deadline-or-None); spread evenly but
                # never past a unit's deadline (attn-unit index by which its
                # output is consumed)
                n_att, n_fill = len(attn), len(filler)
                pos = []
                for i, (u, dl) in enumerate(filler):
                    pt = (i + 1) * n_att // (n_fill + 1)
                    if dl is not None:
                        pt = min(pt, dl)
                    pos.append(pt)
                for i in range(1, n_fill):
                    pos[i] = max(pos[i], pos[i - 1])
                out = []
                fi = 0
                for i, u in enumerate(attn):
                    while fi < n_fill and pos[fi] <= i:
                        out.append(filler[fi][0])
                        fi += 1
                    out.append(u)
                out.extend(f[0] for f in filler[fi:])
                return out

            def kv_deadlines(t):
                # deadlines (attn-unit index in phase t) for stage-t K/V
                # fillers: sweep p first touches kT[p] chunk 4t at its
                # sc(4t); vaug[4t+j] at sweep 0's pv(4t+j)
                n = 4 * (t + 1)
                slen = 2 * n + 1
                ks = [p * slen + max(0, 2 * (4 * t) - 1) for p in range(NPAIR)]
                vs = []
                for j in range(4):
                    kc = 4 * t + j
                    vs.append(2 * kc + 2 if kc <= n - 2 else 2 * n - 1)
                return ks, vs

            # ================= schedule =================
            # pre-phase: stage-0 projections (evacs on ACT; it is idle here).
            # Issue the first loads from different engines in parallel — each
            # dma_start costs ~0.6us of issue time on its engine.
            wtiles["v"] = load_w(wv)
            get_xts("v", 0, eng=nc.gpsimd)
            wtiles["k"] = load_w(wk, eng=nc.scalar)
            wtiles["q"] = load_w(wq, eng=nc.scalar)
            prepools = [projp, pvp, pvp, pvp]
            for s in range(4):
                v_unit(0, s, "act", pool=prepools[s % 4])()
            get_xts("k", 0)
            for m in range(NPAIR):
                kq_unit("k", kT, 0, m, "act", pool=prepools[m % 4])()
            get_xts("q", 0, eng=nc.gpsimd)
            for m in range(NPAIR):
                kq_unit("q", qT, 0, m, "act", pool=prepools[m % 4])()
            load_wo()

            def proj_group(which, dst, t):
                if which == "v":
                    return [v_unit(t, s, "dve") for s in range(4)]
                return [kq_unit(which, dst, t, m, "dve") for m in range(NPAIR)]

            # filler assignments per qb phase, interleaved by deadline:
            # Q(t) must land in phase t-1; K(t)/V(t) are consumed mid-phase t;
            # all wo chunks go to the EXP-heaviest last phase.
            def kv_fillers(t, include_k=True):
                ks, vs = kv_deadlines(t)
                kus = [kq_unit("k", kT, t, m, "dve") for m in range(NPAIR)]
                vus = [v_unit(t, j, "dve") for j in range(4)]
                units = [(vus[j], vs[j]) for j in range(4)]
                if include_k:
                    units += [(kus[m], ks[m]) for m in range(NPAIR)]
                return sorted(units, key=lambda x: x[1])

            fillers = {
                0: [(u, None) for u in
                    [kq_unit("q", qT, 1, m, "act") for m in range(NPAIR)]],
                1: kv_fillers(1) + [(u, None) for u in proj_group("q", qT, 2)],
                2: kv_fillers(2) + [(u, None) for u in proj_group("q", qT, 3)],
                3: kv_fillers(3)
                   + [(wo_unit(t16), None) for t16 in range(0, 12)],
            }

            # x prefetches: stage-t K/V one phase ahead; Q(t) at the start
            # of the phase whose tail projects it
            get_xts("q", 1)
            phase_xts = {
                0: [("k", 1), ("v", 1)],
                1: [("k", 2), ("v", 2), ("q", 2)],
                2: [("k", 3), ("v", 3), ("q", 3)],
                3: [],
            }

            # tail: last q-block's wo, with c=0..2 partial accumulation
            # issued BEFORE the final pair's norm so the PE stays busy (and
            # warm) under the last normalization chain.
            tail_ps = {}
            tail_ot = {}

            def wo_partial(t16, oh, pool, tag):
                def run():
                    if pool is stp:
                        full = pool.tile([128, 2, 512], F32, tag=tag, name="wtail")
                        ps = full[:, 0, :]
                        tail_ps[("dummy", t16, oh)] = full[:, 1, :]
                    else:
                        ps = pool.tile([128, 512], F32, tag=tag, name="wtail")
                    for c in range(3):
                        nc.tensor.matmul(
                            ps,
                            lhsT=ctxT[c][:, t16 * 128:(t16 + 1) * 128],
                            rhs=wo_sb[c][:, oh * 512:(oh + 1) * 512],
                            start=(c == 0),
                            stop=False,
                        )
                    tail_ps[(t16, oh)] = ps
                return run

            def ham_filler(n):
                # dummy matmuls into a spare PSUM region: keep the PE clock
                # warm across the final normalization chain
                def run():
                    spare = tail_ps[("dummy", 12, 0)]
                    for _ in range(n):
                        nc.tensor.matmul(
                            spare, lhsT=wu[:, 0:128], rhs=qT[0][:, 0:512],
                            start=True, stop=True,
                        )
                return run

            def wo_finish(t16, oh, last=False):
                def run():
                    ps = tail_ps.pop((t16, oh))
                    nc.tensor.matmul(
                        ps,
                        lhsT=ctxT[3][:, t16 * 128:(t16 + 1) * 128],
                        rhs=wo_sb[3][:, oh * 512:(oh + 1) * 512],
                        start=False,
                        stop=True,
                    )
                    if t16 not in tail_ot:
                        tail_ot[t16] = outsb.tile([128, D], F32, tag="ot", name="ottile")
                    ot = tail_ot[t16]
                    nc.vector.tensor_copy(ot[:, oh * 512:(oh + 1) * 512], ps)
                    if oh == 1:
                        rows = slice(t16 * 128, (t16 + 1) * 128)
                        nc.sync.dma_start(
                            out=outp[rows, 0:384], in_=ot[:, 0:384]
                        )
                        nc.gpsimd.dma_start(
                            out=outp[rows, 384:704], in_=ot[:, 384:704]
                        )
                        nc.scalar.dma_start(
                            out=outp[rows, 704:D], in_=ot[:, 704:D]
                        )
                return run

            for qb in range(TQ):
                attn = []
                for p in range(NPAIR):
                    units = sweep_units(qb, p)
                    if qb == 3 and p == 3:
                        # wedge the partial wo units between norm_a and
                        # norm_b; dummy matmuls keep the PE warm while the
                        # final norm chain runs
                        units = units[:-1] + [
                            wo_partial(12, 0, stp, "st"),
                            wo_partial(12, 1, stp, "st"),
                            wo_partial(13, 0, projp, "pp"),
                        ] + units[-1:] + [ham_filler(16)]
                    attn.extend(units)
                pf = [(prefetch_unit(which, t, d), None)
                      for which, t in phase_xts[qb] for d in range(DCH)]
                for u in weave(attn, fillers[qb] + pf):
                    u()

            # finalize the tail: c=3 rows + evac, then the remaining chunks
            wo_finish(12, 0)()
            wo_finish(12, 1)()
            wo_finish(13, 0)()
            wo_partial(13, 1, pvp, "pv")()
            wo_finish(13, 1)()
            for t16 in (14, 15):
                for oh in range(2):
                    wo_partial(t16, oh, pvp if oh == 0 else projp, "pv" if oh == 0 else "pp")()
                    wo_finish(t16, oh)()

    nc.compile()
    return nc


_CACHE = {}


def _get_nc(L):
    if L not in _CACHE:
        _CACHE[L] = build(L)
    return _CACHE[L]


def make_in_maps(query, key, value, attention_mask, Wq, Wk, Wv, Wo):
    import ml_dtypes

    B, L, _ = query.shape
    scale = np.float32(1.0 / math.sqrt(DH))
    bf = lambda a: np.ascontiguousarray(np.asarray(a, np.float32)).astype(
        ml_dtypes.bfloat16
    )
    xqT = [bf(np.asarray(query[b]).T) for b in range(B)]
    xkT = [bf(np.asarray(key[b]).T) for b in range(B)]
    xvT = [bf(np.asarray(value[b]).T) for b in range(B)]
    kk, qq = np.meshgrid(np.arange(128), np.arange(128), indexing="ij")
    tri = np.ascontiguousarray(
        np.where(kk > qq, np.float32(NEG), np.float32(0.0)).astype(np.float32)
    )
    in_maps = []
    for core in range(2 * B):
        b, hg = divmod(core, 2)
        sl = slice(hg * HG, (hg + 1) * HG)
        m2 = np.asarray(attention_mask[b]).astype(np.float32).reshape(-1, 128).T
        mc = np.ascontiguousarray(
            np.repeat(m2[:, :, None], NH, 2).reshape(128, -1), dtype=np.float32
        )
        in_maps.append({
            "xq_t": xqT[b],
            "xk_t": xkT[b],
            "xv_t": xvT[b],
            "wq_t": bf(np.asarray(Wq, np.float32)[sl, :].T * scale),
            "wk_t": bf(np.asarray(Wk, np.float32)[sl, :].T),
            "wv_t": bf(np.asarray(Wv, np.float32)[sl, :].T),
            "wo_t": bf(np.asarray(Wo, np.float32)[:, sl].T),
            "maskcol": mc,
            "trimask": bf(tri),
            "ident": bf(np.eye(128, dtype=np.float32)),
        })
    return in_maps


def kernel(query, key, value, attention_mask, Wq, Wk, Wv, Wo, _res_hook=None):
    B, L, D_ = query.shape
    nc = _get_nc(L)
    in_maps = make_in_maps(query, key, value, attention_mask, Wq, Wk, Wv, Wo)
    res = bass_utils.run_bass_kernel_spmd(nc, in_maps, core_ids=list(range(8)))
    if _res_hook is not None:
        _res_hook(res)
    out = np.empty((B, L, D_), np.float32)
    for b in range(B):
        out[b] = res.results[2 * b]["outp"] + res.results[2 * b + 1]["outp"]
    return out


# revision 44
# speedup vs baseline: 1.0112x; 1.0112x over previous
"""Multi-head causal attention (B=4, L=2048, D=1024, H=16) on 8 trn2 cores.

Sharding: (batch, head-group) grid — core c handles batch c//2, heads
(c%2)*8..(c%2)*8+8.  Each core projects Q/K/V for its 8 heads, runs causal
attention, and computes a partial output projection; the host sums the two
head-group partials per batch.

Per-core layouts (host prepares transposed inputs so every matmul contracts
over the partition dim):
  xq_t/xk_t/xv_t [D, L]   : x.T            (rhs / lhsT of projections)
  wq_t/wk_t/wv_t [D, 512] : W_slice.T      (wq pre-scaled by 1/sqrt(dh))
  wo_t           [512, D] : Wo_slice.T
  qT/kT pair tiles [128, L]: rows 0-63 head 2p, 64-127 head 2p+1 (dh on P)
  v_aug [128, 8, 65]      : per 128-token chunk; [:, h, 0:64]=V, [:, h, 64]=key mask
  scores ST [k(P), q(F)]  : transposed scores -> softmax sum via matmul's
                            extra mask column (pv row 64), no P-transposes.

Schedule: projections, attention sweeps and output-projection chunks are
woven into one issue stream so the Scalar engine's EXP (the secondary
bottleneck) always overlaps PE matmuls.  Attention runs as per-(qb, pair)
sweeps with lag-1 PV issue; proj/wo units are inserted as PE filler inside
each sweep.  PSUM: 2x score double-buffer (4 banks) + 3 pv accumulators +
1 proj/wo bank = 8.
"""

import math
from contextlib import ExitStack

import numpy as np

import concourse.bass as bass
import concourse.tile as tile
from concourse import bacc, mybir
from concourse import bass_utils

D = 1024  # model dim
HG = 512  # head dims per core (8 heads x 64)
NH = 8    # heads per core
DH = 64
NPAIR = 4  # head pairs per core
NEG = -1.0e30

F32 = mybir.dt.float32
BF16 = mybir.dt.bfloat16
EXP = mybir.ActivationFunctionType.Exp
COPY = mybir.ActivationFunctionType.Copy


def build(L=2048):
    TQ = L // 512    # 512-token q-blocks
    T16 = L // 128   # 128-token chunks
    DCH = D // 128   # contraction chunks for projections

    nc = bacc.Bacc("TRN2", target_bir_lowering=False, debug=False, num_devices=8)

    xq = nc.dram_tensor("xq_t", [D, L], BF16, kind="ExternalInput").ap()
    xk = nc.dram_tensor("xk_t", [D, L], BF16, kind="ExternalInput").ap()
    xv = nc.dram_tensor("xv_t", [D, L], BF16, kind="ExternalInput").ap()
    wq = nc.dram_tensor("wq_t", [D, HG], BF16, kind="ExternalInput").ap()
    wk = nc.dram_tensor("wk_t", [D, HG], BF16, kind="ExternalInput").ap()
    wv = nc.dram_tensor("wv_t", [D, HG], BF16, kind="ExternalInput").ap()
    wo = nc.dram_tensor("wo_t", [HG, D], BF16, kind="ExternalInput").ap()
    mcol = nc.dram_tensor("maskcol", [128, (L // 128) * NH], F32, kind="ExternalInput").ap()
    trim = nc.dram_tensor("trimask", [128, 128], BF16, kind="ExternalInput").ap()
    iden = nc.dram_tensor("ident", [128, 128], BF16, kind="ExternalInput").ap()
    outp = nc.dram_tensor("outp", [L, D], F32, kind="ExternalOutput").ap()

    with ExitStack() as ctx:
        tc = ctx.enter_context(tile.TileContext(nc))

        # ---- persistent tiles ----
        singles = ctx.enter_context(tc.tile_pool(name="singles", bufs=1))
        qT = [singles.tile([128, L], BF16, tag=f"qT{p}", name=f"qT{p}") for p in range(NPAIR)]
        kT = [singles.tile([128, L], BF16, tag=f"kT{p}", name=f"kT{p}") for p in range(NPAIR)]
        vaug = [singles.tile([128, NH, DH + 1], BF16, tag=f"vaug{t}", name=f"vaug{t}") for t in range(T16)]
        ctxT = [singles.tile([128, L], BF16, tag=f"ctxT{p}", name=f"ctxT{p}") for p in range(NPAIR)]
        mc_sb = singles.tile([128, T16, NH], F32, tag="mc")
        tri_sb = singles.tile([128, 128], BF16, tag="tri")
        id_sb = singles.tile([128, 128], BF16, tag="ident")

        nc.sync.dma_start(out=mc_sb, in_=mcol.rearrange("p (t h) -> p t h", h=NH))
        nc.sync.dma_start(out=tri_sb, in_=trim)
        nc.sync.dma_start(out=id_sb, in_=iden)

        with (
            tc.tile_pool(name="xt", bufs=26) as xtp,
            tc.tile_pool(name="w", bufs=3 * DCH) as wp,
            tc.tile_pool(name="stp", bufs=2, space="PSUM") as stp,
            tc.tile_pool(name="pvp", bufs=3, space="PSUM") as pvp,
            tc.tile_pool(name="projp", bufs=1, space="PSUM") as projp,
            tc.tile_pool(name="expp", bufs=5) as expp,
            tc.tile_pool(name="recp", bufs=2) as recp,
            tc.tile_pool(name="bcsb", bufs=2) as bcsbp,
            tc.tile_pool(name="tmpb", bufs=4) as tmpbp,
            tc.tile_pool(name="wop", bufs=NPAIR) as wop,
            tc.tile_pool(name="outp_sb", bufs=3) as outsb,
        ):
            # PE warm-up (HAM) while the first DMAs land: a few dummy
            # matmuls with no data dependencies.
            wu = singles.tile([128, 256], BF16, tag="warm")
            nc.vector.memset(wu, 0.0)
            ones = singles.tile([128, DH], F32, tag="ones")
            nc.vector.memset(ones, 1.0)
            wups = stp.tile([128, 2, 512], F32, tag="st", name="wupstile")
            for _ in range(38):
                nc.tensor.matmul(
                    wups[:, 0, 0:256], lhsT=wu[:, 0:128], rhs=wu, start=True, stop=True
                )

            def load_w(wdram, eng=None):
                eng = eng or nc.sync
                tiles = [wp.tile([128, HG], BF16, tag="w", name="wtile") for _ in range(DCH)]
                for d in range(DCH):
                    eng.dma_start(out=tiles[d], in_=wdram[d * 128:(d + 1) * 128, :])
                return tiles

            xts_cache = {}

            def _xts_entry(which, t):
                key = (which, t)
                if key not in xts_cache:
                    xts_cache[key] = {
                        "tiles": [xtp.tile([128, 512], BF16, tag="xt", name="xtile")
                                  for _ in range(DCH)],
                        "issued": [False] * DCH,
                    }
                return xts_cache[key]

            def _issue_xd(which, t, d, eng):
                ent = _xts_entry(which, t)
                if not ent["issued"][d]:
                    xd = {"q": xq, "k": xk, "v": xv}[which]
                    eng.dma_start(
                        out=ent["tiles"][d],
                        in_=xd[d * 128:(d + 1) * 128, t * 512:(t + 1) * 512],
                    )
                    ent["issued"][d] = True

            def get_xts(which, t, eng=None):
                e = eng or nc.sync
                for d in range(DCH):
                    _issue_xd(which, t, d, e)
                return _xts_entry(which, t)["tiles"]

            def prefetch_unit(which, t, d):
                # one tile's input DMA, woven into the filler stream so the
                # descriptor burst never floods all queues at once
                def run():
                    _issue_xd(which, t, d, nc.sync)
                return run

            wtiles = {}

            # ---- unit builders (each returns a closure doing ~1-2us of PE) ----
            def kq_unit(which, dst, t, m, evac, pool=None):
                # dst[m][:, t*512:+512] = (W.T chunk m).T @ xT
                def run():
                    xts = get_xts(which, t)
                    wt = wtiles[which]
                    pl = pool or projp
                    ps = pl.tile([128, 512], F32, tag="pp" if pl is projp else "pv",
                                 name="psproj")
                    for d in range(DCH):
                        nc.tensor.matmul(
                            ps,
                            lhsT=wt[d][:, m * 128:(m + 1) * 128],
                            rhs=xts[d],
                            start=(d == 0),
                            stop=(d == DCH - 1),
                        )
                    dstap = dst[m][:, t * 512:(t + 1) * 512]
                    if evac == "act":
                        nc.scalar.copy(dstap, ps)
                    else:
                        nc.vector.tensor_copy(dstap, ps)
                return run

            def v_unit(t, s, evac, pool=None):
                # v_aug[t16][:, h, 0:64] = (x @ Wv.T)[tok chunk, head h] * mask
                def run():
                    xts = get_xts("v", t)
                    wt = wtiles["v"]
                    t16 = t * 4 + s
                    pl = pool or projp
                    ps = pl.tile([128, 512], F32, tag="pp" if pl is projp else "pv",
                                 name="psv")
                    for d in range(DCH):
                        nc.tensor.matmul(
                            ps,
                            lhsT=xts[d][:, s * 128:(s + 1) * 128],
                            rhs=wt[d],
                            start=(d == 0),
                            stop=(d == DCH - 1),
                        )
                    src = ps.rearrange("p (h e) -> p h e", h=NH)
                    dstap = vaug[t16][:, :, 0:DH]
                    if evac == "act":
                        nc.scalar.activation(
                            out=dstap, in_=src, func=COPY,
                            scale=mc_sb[:, t16, 0:1],
                        )
                    else:
                        nc.vector.tensor_scalar_mul(dstap, src, mc_sb[:, t16, 0:1])
                    nc.vector.tensor_copy(
                        vaug[t16][:, :, DH:DH + 1], mc_sb[:, t16:t16 + 1, :],
                    )
                return run

            wo_sb = [wop.tile([128, D], BF16, tag="wo", name="wotile") for _ in range(NPAIR)]

            def load_wo():
                for c in range(NPAIR):
                    nc.sync.dma_start(out=wo_sb[c], in_=wo[c * 128:(c + 1) * 128, :])

            def wo_unit(t16, evac="dve"):
                def run():
                    ot = outsb.tile([128, D], F32, tag="ot", name="ottile")
                    for oh in range(2):
                        ps = projp.tile([128, 512], F32, tag="pp", name="potile")
                        for c in range(NPAIR):
                            nc.tensor.matmul(
                                ps,
                                lhsT=ctxT[c][:, t16 * 128:(t16 + 1) * 128],
                                rhs=wo_sb[c][:, oh * 512:(oh + 1) * 512],
                                start=(c == 0),
                                stop=(c == NPAIR - 1),
                            )
                        oap = ot[:, oh * 512:(oh + 1) * 512]
                        if evac == "act":
                            nc.scalar.copy(oap, ps)
                        else:
                            nc.vector.tensor_copy(oap, ps)
                    nc.sync.dma_start(out=outp[t16 * 128:(t16 + 1) * 128, :], in_=ot)
                return run

            # ---- attention sweep over one (qb, pair): list of units with
            # lag-1 PV issue so the PE never waits at an EXP-gated matmul.
            def sweep_units(qb, p):
                nkc = 4 * (qb + 1)
                pv = [pvp.tile([DH + 1, 512], F32, tag="pv", name="pvtile")
                      for _ in range(2)]
                exs = {}

                def score_unit(kc):
                    def run():
                        j = kc - 4 * qb
                        off = j * 128 if j >= 0 else 0
                        st = stp.tile([128, 2, 512], F32, tag="st", name="sttile")
                        for ph in range(2):
                            nc.tensor.matmul(
                                st[:, ph, off:512],
                                lhsT=kT[p][ph * DH:(ph + 1) * DH,
                                           kc * 128:(kc + 1) * 128],
                                rhs=qT[p][ph * DH:(ph + 1) * DH,
                                          qb * 512 + off:(qb + 1) * 512],
                                start=True,
                                stop=(j < 0),
                            )
                        if j >= 0:
                            # causal mask via PE: st += I.T @ tri
                            for ph in range(2):
                                nc.tensor.matmul(
                                    st[:, ph, off:off + 128],
                                    lhsT=id_sb,
                                    rhs=tri_sb,
                                    start=False,
                                    stop=True,
                                )
                        ex = expp.tile([128, 2, 512], BF16, tag="ex", name="extile")
                        nc.scalar.activation(
                            out=ex[:, :, off:512], in_=st[:, :, off:512], func=EXP
                        )
                        exs[kc] = (ex, off)
                    return run

                def pv_unit(kc):
                    def run():
                        ex, off = exs.pop(kc)
                        for ph in range(2):
                            nc.tensor.matmul(
                                pv[ph][:, off:512],
                                lhsT=vaug[kc][:, 2 * p + ph, :],
                                rhs=ex[:, ph, off:512],
                                start=(kc == 0),
                                stop=(kc == nkc - 1),
                            )
                    return run

                nstate = {}
                last_sweep = (qb == 3 and p == 3)

                def norm_a():
                    def run():
                        bcs = bcsbp.tile([DH, 2, 512], F32, tag="bcs", name="bcstile")
                        if last_sweep:
                            # latency-critical: PE ones-outer-product
                            # broadcast + wide reciprocal
                            s64 = recp.tile([128, 2, 512], F32, tag="rec", name="s64tile")
                            for ph in range(2):
                                nc.vector.tensor_copy(
                                    s64[DH:DH + 1, ph, :], pv[ph][DH:DH + 1, :]
                                )
                            dps = stp.tile([128, 2, 512], F32, tag="st", name="dbcast")
                            for ph in range(2):
                                nc.tensor.matmul(
                                    dps[0:DH, ph, :],
                                    lhsT=ones[DH:DH + 1, :],
                                    rhs=s64[DH:DH + 1, ph, :],
                                    start=True,
                                    stop=True,
                                )
                            nc.vector.reciprocal_approx_fast(
                                out=bcs, in_=dps[0:DH, :, :]
                            )
                        else:
                            # two per-ph chains pipelined across DVE/GPS:
                            # copy -> DMA to partition 0 -> reciprocal ->
                            # GPSIMD broadcast
                            for ph in range(2):
                                s64 = recp.tile([128, 512], F32, tag="rec", name="s64tile")
                                nc.vector.tensor_copy(
                                    s64[DH:DH + 1, :], pv[ph][DH:DH + 1, :]
                                )
                                srow = recp.tile([1, 512], F32, tag="srow", name="srowtile")
                                nc.gpsimd.dma_start(out=srow, in_=s64[DH:DH + 1, :])
                                rcp = recp.tile([1, 512], F32, tag="rcp", name="rcptile")
                                nc.vector.reciprocal_approx_fast(out=rcp, in_=srow)
                                nc.gpsimd.partition_broadcast(
                                    bcs[:, ph, :], rcp, channels=DH
                                )
                        nstate["bcs"] = bcs
                    return run

                def norm_b():
                    def run():
                        # ctxT[p][:, qb block] = pv[0:64] * (1/denom); ph0
                        # first — its slot gates the next sweep's PV via the
                        # psum pool ring
                        bcs = nstate.pop("bcs")
                        nc.vector.tensor_mul(
                            ctxT[p][0:DH, qb * 512:(qb + 1) * 512],
                            pv[0][0:DH, :],
                            bcs[:, 0, :],
                        )
                        tb = tmpbp.tile([DH, 512], BF16, tag="tb", name="tbtile")
                        nc.vector.tensor_mul(tb, pv[1][0:DH, :], bcs[:, 1, :])
                        shift_eng = nc.sync if last_sweep else nc.gpsimd
                        shift_eng.dma_start(
                            out=ctxT[p][DH:128, qb * 512:(qb + 1) * 512],
                            in_=tb,
                        )
                    return run

                def gap_dummy(n):
                    # dummy matmuls in the proj/wo psum slot: execute while
                    # this sweep's first PV waits on the pv-ring WAR, keeping
                    # the HAM activity window busy (qb0 sweeps are shorter
                    # than the norm chain)
                    def run():
                        ps = projp.tile([128, 512], F32, tag="pp", name="gapdummy")
                        for _ in range(n):
                            nc.tensor.matmul(
                                ps, lhsT=wu[:, 0:128], rhs=qT[0][:, 0:512],
                                start=True, stop=True,
                            )
                    return run

                units = [score_unit(0)]
                for kc in range(1, nkc):
                    units.append(score_unit(kc))
                    units.append(pv_unit(kc - 1))
                units.append(pv_unit(nkc - 1))
                units.append(norm_a())
                units.append(norm_b())
                return units

            def weave(attn, filler):
                # filler: list of (unit, deadline-or-None); spread evenly but
                # never past a unit's deadline (attn-unit index by which its
                # output is consumed)
                n_att, n_fill = len(attn), len(filler)
                pos = []
                for i, (u, dl) in enumerate(filler):
                    pt = (i + 1) * n_att // (n_fill + 1)
                    if i < 2:
                        # cover the phase-boundary PV-ring stall window
                        pt = min(pt, 1 + 2 * i)
                    if dl is not None:
                        pt = min(pt, dl)
                    pos.append(pt)
                for i in range(1, n_fill):
                    pos[i] = max(pos[i], pos[i - 1])
                out = []
                fi = 0
                for i, u in enumerate(attn):
                    while fi < n_fill and pos[fi] <= i:
                        out.append(filler[fi][0])
                        fi += 1
                    out.append(u)
                out.extend(f[0] for f in filler[fi:])
                return out

            def kv_deadlines(t):
                # deadlines (attn-unit index in phase t) for stage-t K/V
                # fillers: sweep p first touches kT[p] chunk 4t at its
                # sc(4t); vaug[4t+j] at sweep 0's pv(4t+j)
                n = 4 * (t + 1)
                slen = 2 * n + 1
                ks = [p * slen + max(0, 2 * (4 * t) - 1) for p in range(NPAIR)]
                vs = []
                for j in range(4):
                    kc = 4 * t + j
                    vs.append(2 * kc + 2 if kc <= n - 2 else 2 * n - 1)
                return ks, vs

            # ================= schedule =================
            # pre-phase: stage-0 projections (evacs on ACT; it is idle here).
            # Issue the first loads from different engines in parallel — each
            # dma_start costs ~0.6us of issue time on its engine.
            wtiles["v"] = load_w(wv)
            get_xts("v", 0, eng=nc.gpsimd)
            wtiles["k"] = load_w(wk, eng=nc.scalar)
            wtiles["q"] = load_w(wq, eng=nc.scalar)
            prepools = [projp, pvp, pvp, pvp]
            for s in range(4):
                v_unit(0, s, "act", pool=prepools[s % 4])()
            get_xts("k", 0)
            for m in range(NPAIR):
                kq_unit("k", kT, 0, m, "act", pool=prepools[m % 4])()
            get_xts("q", 0, eng=nc.gpsimd)
            for m in range(NPAIR):
                kq_unit("q", qT, 0, m, "act", pool=prepools[m % 4])()
            load_wo()

            def proj_group(which, dst, t):
                if which == "v":
                    return [v_unit(t, s, "dve") for s in range(4)]
                return [kq_unit(which, dst, t, m, "dve") for m in range(NPAIR)]

            # filler assignments per qb phase, interleaved by deadline:
            # Q(t) must land in phase t-1; K(t)/V(t) are consumed mid-phase t;
            # all wo chunks go to the EXP-heaviest last phase.
            def kv_fillers(t, include_k=True):
                ks, vs = kv_deadlines(t)
                kus = [kq_unit("k", kT, t, m, "dve") for m in range(NPAIR)]
                vus = [v_unit(t, j, "dve") for j in range(4)]
                units = [(vus[j], vs[j]) for j in range(4)]
                if include_k:
                    units += [(kus[m], ks[m]) for m in range(NPAIR)]
                return sorted(units, key=lambda x: x[1])

            fillers = {
                0: [(u, None) for u in
                    [kq_unit("q", qT, 1, m, "act") for m in range(NPAIR)]],
                1: kv_fillers(1) + [(u, None) for u in proj_group("q", qT, 2)],
                2: kv_fillers(2) + [(u, None) for u in proj_group("q", qT, 3)],
                3: kv_fillers(3)
                   + [(wo_unit(t16), None) for t16 in range(0, 12)],
            }

            # x prefetches: stage-t K/V one phase ahead; Q(t) at the start
            # of the phase whose tail projects it
            get_xts("q", 1)
            phase_xts = {
                0: [("k", 1), ("v", 1)],
                1: [("k", 2), ("v", 2), ("q", 2)],
                2: [("k", 3), ("v", 3), ("q", 3)],
                3: [],
            }

            # tail: last q-block's wo, with c=0..2 partial accumulation
            # issued BEFORE the final pair's norm so the PE stays busy (and
            # warm) under the last normalization chain.
            tail_ps = {}
            tail_ot = {}

            def wo_partial(t16, oh, pool, tag):
                def run():
                    if pool is stp:
                        full = pool.tile([128, 2, 512], F32, tag=tag, name="wtail")
                        ps = full[:, 0, :]
                        tail_ps[("dummy", t16, oh)] = full[:, 1, :]
                    else:
                        ps = pool.tile([128, 512], F32, tag=tag, name="wtail")
                    for c in range(3):
                        nc.tensor.matmul(
                            ps,
                            lhsT=ctxT[c][:, t16 * 128:(t16 + 1) * 128],
                            rhs=wo_sb[c][:, oh * 512:(oh + 1) * 512],
                            start=(c == 0),
                            stop=False,
                        )
                    tail_ps[(t16, oh)] = ps
                return run

            def ham_filler(n):
                # dummy matmuls into a spare PSUM region: keep the PE clock
                # warm across the final normalization chain
                def run():
                    spare = tail_ps[("dummy", 12, 0)]
                    for _ in range(n):
                        nc.tensor.matmul(
                            spare, lhsT=wu[:, 0:128], rhs=qT[0][:, 0:512],
                            start=True, stop=True,
                        )
                return run

            def wo_finish(t16, oh, last=False):
                def run():
                    ps = tail_ps.pop((t16, oh))
                    nc.tensor.matmul(
                        ps,
                        lhsT=ctxT[3][:, t16 * 128:(t16 + 1) * 128],
                        rhs=wo_sb[3][:, oh * 512:(oh + 1) * 512],
                        start=False,
                        stop=True,
                    )
                    if t16 not in tail_ot:
                        tail_ot[t16] = outsb.tile([128, D], F32, tag="ot", name="ottile")
                    ot = tail_ot[t16]
                    nc.vector.tensor_copy(ot[:, oh * 512:(oh + 1) * 512], ps)
                    if oh == 1:
                        rows = slice(t16 * 128, (t16 + 1) * 128)
                        nc.sync.dma_start(
                            out=outp[rows, 0:384], in_=ot[:, 0:384]
                        )
                        nc.gpsimd.dma_start(
                            out=outp[rows, 384:704], in_=ot[:, 384:704]
                        )
                        nc.scalar.dma_start(
                            out=outp[rows, 704:D], in_=ot[:, 704:D]
                        )
                return run

            for qb in range(TQ):
                attn = []
                for p in range(NPAIR):
                    units = sweep_units(qb, p)
                    if qb == 3 and p == 3:
                        # wedge the partial wo units between norm_a and
                        # norm_b; dummy matmuls keep the PE warm while the
                        # final norm chain runs
                        units = units[:-1] + [
                            wo_partial(12, 0, stp, "st"),
                            wo_partial(12, 1, stp, "st"),
                            wo_partial(13, 0, projp, "pp"),
                        ] + units[-1:] + [ham_filler(16)]
                    attn.extend(units)
                pf = [(prefetch_unit(which, t, d), None)
                      for which, t in phase_xts[qb] for d in range(DCH)]
                # phase 0: prefetches first so the Q1 proj fillers land late,
                # after their own input tiles have arrived
                flist = (pf + fillers[qb]) if qb == 0 else (fillers[qb] + pf)
                for u in weave(attn, flist):
                    u()

            # finalize the tail: c=3 rows + evac, then the remaining chunks
            wo_finish(12, 0)()
            wo_finish(12, 1)()
            wo_finish(13, 0)()
            wo_partial(13, 1, pvp, "pv")()
            wo_finish(13, 1)()
            for t16 in (14, 15):
                for oh in range(2):
                    wo_partial(t16, oh, pvp if oh == 0 else projp, "pv" if oh == 0 else "pp")()
                    wo_finish(t16, oh)()

    nc.compile()
    return nc


_CACHE = {}


def _get_nc(L):
    if L not in _CACHE:
        _CACHE[L] = build(L)
    return _CACHE[L]


def make_in_maps(query, key, value, attention_mask, Wq, Wk, Wv, Wo):
    import ml_dtypes

    B, L, _ = query.shape
    scale = np.float32(1.0 / math.sqrt(DH))
    bf = lambda a: np.ascontiguousarray(np.asarray(a, np.float32)).astype(
        ml_dtypes.bfloat16
    )
    xqT = [bf(np.asarray(query[b]).T) for b in range(B)]
    xkT = [bf(np.asarray(key[b]).T) for b in range(B)]
    xvT = [bf(np.asarray(value[b]).T) for b in range(B)]
    kk, qq = np.meshgrid(np.arange(128), np.arange(128), indexing="ij")
    tri = np.ascontiguousarray(
        np.where(kk > qq, np.float32(NEG), np.float32(0.0)).astype(np.float32)
    )
    in_maps = []
    for core in range(2 * B):
        b, hg = divmod(core, 2)
        sl = slice(hg * HG, (hg + 1) * HG)
        m2 = np.asarray(attention_mask[b]).astype(np.float32).reshape(-1, 128).T
        mc = np.ascontiguousarray(
            np.repeat(m2[:, :, None], NH, 2).reshape(128, -1), dtype=np.float32
        )
        in_maps.append({
            "xq_t": xqT[b],
            "xk_t": xkT[b],
            "xv_t": xvT[b],
            "wq_t": bf(np.asarray(Wq, np.float32)[sl, :].T * scale),
            "wk_t": bf(np.asarray(Wk, np.float32)[sl, :].T),
            "wv_t": bf(np.asarray(Wv, np.float32)[sl, :].T),
            "wo_t": bf(np.asarray(Wo, np.float32)[:, sl].T),
            "maskcol": mc,
            "trimask": bf(tri),
            "ident": bf(np.eye(128, dtype=np.float32)),
        })
    return in_maps


def kernel(query, key, value, attention_mask, Wq, Wk, Wv, Wo, _res_hook=None):
    B, L, D_ = query.shape
    nc = _get_nc(L)
    in_maps = make_in_maps(query, key, value, attention_mask, Wq, Wk, Wv, Wo)
    res = bass_utils.run_bass_kernel_spmd(nc, in_maps, core_ids=list(range(8)))
    if _res_hook is not None:
        _res_hook(res)
    out = np.empty((B, L, D_), np.float32)
    for b in range(B):
        out[b] = res.results[2 * b]["outp"] + res.results[2 * b + 1]["outp"]
    return out
